# revision 10
# baseline (speedup 1.0000x reference)
"""Trainium2 Bass kernel for nn_CNNQNetwork (dense_cnn).

The reference network applies 7 small convs to a fixed 4x4x16 input with
VALID padding, concatenates the relu'd outputs (3648 features), then a
3-layer MLP (3648 -> 512 -> 128 -> 4).  Because the spatial input is tiny
and fixed, the whole conv+concat stage is one linear map of the flattened
input: combined = relu(x_flat @ Wc.T + bc) with Wc [3648, 256] assembled
on the host from the conv weights.  So the device kernel is a 4-layer MLP:

    256 -> 3648 (relu) -> 512 (relu) -> 128 (relu) -> 4

Sharding: pure data parallel over 8 NeuronCores (4096 samples each),
weights replicated.  Activations are kept feature-major on-chip
(partitions = features, free dim = batch) so every layer is a natural
lhsT.T @ rhs matmul with no on-chip transposes; the host pre-transposes x
and post-transposes the [4, B] output.  Matmuls run in float32r (fp32
operands truncated to ~fp22 at the PE) which streams at full PE rate for
moving dims >= 256, with fp32 PSUM accumulation.
"""

import numpy as np

try:
    # Persistent XLA executable cache: skips the multi-minute NeuronCC
    # compile on repeat runs of the identical program in this container.
    import jax as _jax

    _jax.config.update("jax_compilation_cache_dir", "/tmp/jax_cache")
    _jax.config.update("jax_persistent_cache_min_compile_time_secs", 2)
    _jax.config.update("jax_persistent_cache_min_entry_size_bytes", 0)
except Exception:
    pass

import concourse.bass as bass
import concourse.bacc as bacc
import concourse.mybir as mybir
import concourse.tile as tile
from concourse.bass import ts
from concourse.bass_utils import run_bass_kernel_spmd

N_CORES = 8
B = 32768
B_LOC = B // N_CORES  # 4096
NB = 512  # batch tile (matmul moving dim)
BT = B_LOC // NB  # 8 batch tiles per core
P = 128
F_IN = 256  # 16*4*4 flattened input features
K1 = F_IN // P  # 2
H1 = 3712  # 3648 padded up to 29*128
M1 = H1 // P  # 29
H2 = 512
M2 = H2 // P  # 4
H3 = 128
NA = 4  # num actions

F32 = mybir.dt.float32
F32R = mybir.dt.float32r

KERNELS = [(1, 2), (2, 1), (1, 3), (3, 1), (1, 4), (4, 1), (2, 2)]

_PROGRAM_CACHE = {}


def _build_dense_first_layer(ws, bs):
    """Collapse the 7 convs into one dense [H1, 256] matrix + bias [H1]."""
    Wc = np.zeros((H1, F_IN), np.float32)
    bc = np.zeros((H1,), np.float32)
    off = 0
    for (kh, kw), w, b in zip(KERNELS, ws, bs):
        oh, ow = 5 - kh, 5 - kw
        blk = np.zeros((64, oh, ow, 16, 4, 4), np.float32)
        for pi in range(oh):
            for pj in range(ow):
                blk[:, pi, pj, :, pi : pi + kh, pj : pj + kw] = w
        n = 64 * oh * ow
        Wc[off : off + n] = blk.reshape(n, F_IN)
        bc[off : off + n] = np.repeat(np.asarray(b, np.float32), oh * ow)
        off += n
    assert off == 3648
    return Wc, bc


def _build_program(repeat=1):
    nc = bacc.Bacc(None, target_bir_lowering=False)
    x_d = nc.declare_dram_parameter("x", [K1, P, B_LOC], F32R, isOutput=False)
    wct_d = nc.declare_dram_parameter("wct", [K1, P, H1], F32R, isOutput=False)
    bc_d = nc.declare_dram_parameter("bc", [P, M1], F32, isOutput=False)
    fw0_d = nc.declare_dram_parameter("fw0t", [M1, P, H2], F32R, isOutput=False)
    fb0_d = nc.declare_dram_parameter("fb0", [P, M2], F32, isOutput=False)
    fw1_d = nc.declare_dram_parameter("fw1t", [M2, P, H3], F32R, isOutput=False)
    fb1_d = nc.declare_dram_parameter("fb1", [P, 1], F32, isOutput=False)
    fw2_d = nc.declare_dram_parameter("fw2t", [P, NA], F32R, isOutput=False)
    fb2_d = nc.declare_dram_parameter("fb2", [NA, 1], F32, isOutput=False)
    out_d = nc.declare_dram_parameter("out", [NA, B_LOC], F32, isOutput=True)

    RELU = mybir.ActivationFunctionType.Relu
    ADD = mybir.AluOpType.add
    MAX = mybir.AluOpType.max

    with tile.TileContext(nc) as tc:
        with (
            tc.tile_pool(name="wpool", bufs=1) as wpool,
            tc.tile_pool(name="xpool", bufs=2) as xpool,
            tc.tile_pool(name="a1pool", bufs=1) as a1pool,
            tc.tile_pool(name="apool", bufs=2) as apool,
            tc.tile_pool(name="opool", bufs=2) as opool,
            tc.tile_pool(name="pspool", bufs=4, space="PSUM") as pspool,
            tc.tile_pool(name="ps4pool", bufs=2, space="PSUM") as ps4pool,
        ):
            # --- load all (replicated) weights once; they stay resident ---
            wc = wpool.tile([P, K1, H1], F32R)
            for k in range(K1):
                nc.sync.dma_start(wc[:, k, :], wct_d[k])
            fw0 = wpool.tile([P, M1, H2], F32R)
            for m in range(M1):
                nc.sync.dma_start(fw0[:, m, :], fw0_d[m])
            fw1 = wpool.tile([P, M2, H3], F32R)
            for m in range(M2):
                nc.sync.dma_start(fw1[:, m, :], fw1_d[m])
            fw2 = wpool.tile([P, NA], F32R)
            nc.sync.dma_start(fw2[:], fw2_d[:])
            bc = wpool.tile([P, M1], F32)
            nc.sync.dma_start(bc[:], bc_d[:])
            fb0 = wpool.tile([P, M2], F32)
            nc.sync.dma_start(fb0[:], fb0_d[:])
            fb1 = wpool.tile([P, 1], F32)
            nc.sync.dma_start(fb1[:], fb1_d[:])
            fb2 = wpool.tile([NA, 1], F32)
            nc.sync.dma_start(fb2[:], fb2_d[:])

            def body():
              for t in range(BT):
                xt = xpool.tile([P, K1, NB], F32R, tag="xt")
                for k in range(K1):
                    nc.sync.dma_start(xt[:, k, :], x_d[k, :, ts(t, NB)])

                # L1: a1 = relu(Wc @ x + bc), feature-major [H1, NB]
                a1 = a1pool.tile([P, M1, NB], F32R, tag="a1")
                for m in range(M1):
                    ps = pspool.tile([P, NB], F32, tag="ps")
                    for k in range(K1):
                        nc.tensor.matmul(
                            ps[:],
                            wc[:, k, ts(m, P)],
                            xt[:, k, :],
                            start=(k == 0),
                            stop=(k == K1 - 1),
                        )
                    # split bias+relu between DVE and ACT so neither lags PE
                    if m % 2 == 0:
                        nc.vector.tensor_scalar(
                            a1[:, m, :], ps[:], bc[:, m : m + 1], 0.0, ADD, MAX
                        )
                    else:
                        nc.scalar.activation(
                            a1[:, m, :], ps[:], RELU, bias=bc[:, m : m + 1]
                        )

                # L2: a2 = relu(fw0 @ a1 + fb0), [512, NB]
                a2 = apool.tile([P, M2, NB], F32R, tag="a2")
                for m in range(M2):
                    ps = pspool.tile([P, NB], F32, tag="ps")
                    for k in range(M1):
                        nc.tensor.matmul(
                            ps[:],
                            fw0[:, k, ts(m, P)],
                            a1[:, k, :],
                            start=(k == 0),
                            stop=(k == M1 - 1),
                        )
                    if m % 2 == 0:
                        nc.vector.tensor_scalar(
                            a2[:, m, :], ps[:], fb0[:, m : m + 1], 0.0, ADD, MAX
                        )
                    else:
                        nc.scalar.activation(
                            a2[:, m, :], ps[:], RELU, bias=fb0[:, m : m + 1]
                        )

                # L3: a3 = relu(fw1 @ a2 + fb1), [128, NB]
                a3 = apool.tile([P, NB], F32R, tag="a3")
                ps = pspool.tile([P, NB], F32, tag="ps")
                for k in range(M2):
                    nc.tensor.matmul(
                        ps[:],
                        fw1[:, k, :],
                        a2[:, k, :],
                        start=(k == 0),
                        stop=(k == M2 - 1),
                    )
                nc.scalar.activation(a3[:], ps[:], RELU, bias=fb1[:, 0:1])

                # L4: out = fw2 @ a3 + fb2, [4, NB]
                ps4 = ps4pool.tile([NA, NB], F32, tag="ps4")
                nc.tensor.matmul(
                    ps4[:],
                    fw2[:],
                    a3[:],
                    start=True,
                    stop=True,
                )
                ob = opool.tile([NA, NB], F32, tag="ob")
                nc.vector.tensor_scalar_add(ob[:], ps4[:], fb2[:, 0:1])
                nc.sync.dma_start(out_d[:, ts(t, NB)], ob[:])

            if repeat == 1:
                body()
            else:
                with tc.For_i(0, repeat, 1):
                    body()

    nc.finalize()
    return nc


def pack_inputs(x, ws, bs, fw0, fb0, fw1, fb1, fw2, fb2):
    """Pack full-problem numpy inputs into the per-core DRAM in_maps."""
    x = np.asarray(x, np.float32).reshape(B, F_IN)
    ws = [np.asarray(w, np.float32) for w in ws]
    bs = [np.asarray(b, np.float32) for b in bs]
    fw0 = np.asarray(fw0, np.float32)
    fb0 = np.asarray(fb0, np.float32)
    fw1 = np.asarray(fw1, np.float32)
    fb1 = np.asarray(fb1, np.float32)
    fw2 = np.asarray(fw2, np.float32)
    fb2 = np.asarray(fb2, np.float32)

    Wc, bc = _build_dense_first_layer(ws, bs)

    # host-side packing into the DRAM layouts the kernel expects
    wct_h = np.ascontiguousarray(Wc.T).reshape(K1, P, H1)
    bc_h = np.ascontiguousarray(bc.reshape(M1, P).T)
    fw0_pad = np.zeros((H2, H1), np.float32)
    fw0_pad[:, :3648] = fw0
    fw0t_h = np.ascontiguousarray(fw0_pad.T).reshape(M1, P, H2)
    fb0_h = np.ascontiguousarray(fb0.reshape(M2, P).T)
    fw1t_h = np.ascontiguousarray(fw1.T).reshape(M2, P, H3)
    fb1_h = np.ascontiguousarray(fb1.reshape(1, P).T)
    fw2t_h = np.ascontiguousarray(fw2.T)  # [128, 4]
    fb2_h = np.ascontiguousarray(fb2.reshape(1, NA).T)  # [4, 1]

    shared = {
        "wct": wct_h,
        "bc": bc_h,
        "fw0t": fw0t_h,
        "fb0": fb0_h,
        "fw1t": fw1t_h,
        "fb1": fb1_h,
        "fw2t": fw2t_h,
        "fb2": fb2_h,
    }
    in_maps = []
    for i in range(N_CORES):
        shard = np.ascontiguousarray(
            x[i * B_LOC : (i + 1) * B_LOC].T
        ).reshape(K1, P, B_LOC)
        in_maps.append({"x": shard, **shared})
    return in_maps


def kernel(x, w0, b0, w1, b1, w2, b2, w3, b3, w4, b4, w5, b5, w6, b6,
           fw0, fb0, fw1, fb1, fw2, fb2):
    in_maps = pack_inputs(
        x, (w0, w1, w2, w3, w4, w5, w6), (b0, b1, b2, b3, b4, b5, b6),
        fw0, fb0, fw1, fb1, fw2, fb2,
    )
    if "nc" not in _PROGRAM_CACHE:
        _PROGRAM_CACHE["nc"] = _build_program()
    nc = _PROGRAM_CACHE["nc"]

    res = run_bass_kernel_spmd(nc, in_maps, list(range(N_CORES)))
    out = np.concatenate([r["out"] for r in res.results], axis=1)  # [4, B]
    return np.ascontiguousarray(out.T)


# revision 14
# speedup vs baseline: 1.1248x; 1.1248x over previous
"""Trainium2 Bass kernel for nn_CNNQNetwork (dense_cnn).

The reference network applies 7 small convs to a fixed 4x4x16 input with
VALID padding, concatenates the relu'd outputs (3648 features), then a
3-layer MLP (3648 -> 512 -> 128 -> 4).  Because the spatial input is tiny
and fixed, the conv+concat stage is one linear map of the flattened input,
so the device kernel is a 4-layer MLP:

    256 -> 3712 (relu, padded) -> 512 (relu) -> 128 (relu) -> 4

Sharding: pure data parallel over 8 NeuronCores (4096 samples each),
weights replicated.  Activations are kept feature-major on-chip
(partitions = features, free dim = batch) so every layer is a natural
lhsT.T @ rhs matmul with no on-chip transposes; the host pre-transposes x
and post-transposes the [4, B] output.  Matmuls run in float32r (fp32
operands truncated to ~fp22 at the PE) which streams at full PE rate for
moving dims >= 256, with fp32 PSUM accumulation.

First-layer structure trick: a dense 256-contraction needs 2 matmuls per
128-feature output chunk.  Instead, x is shipped in two spatial layouts
(A: row = (h*4+w)*16+c, B: row = (w*4+h)*16+c).  Output features are
permuted (absorbed into fw0's columns) so each 128-feature chunk covers 2
conv output positions whose input support fits inside ONE 128-row half of
a layout; the chunk is then a single K=128 matmul against that half with
zero-padded weight rows.  Only the 2x2 conv's middle row straddles both
halves (2 matmuls).  31 matmuls/batch-tile instead of 58.
"""

import numpy as np

try:
    # Persistent XLA executable cache: skips the multi-minute NeuronCC
    # compile on repeat runs of the identical program in this container.
    import jax as _jax

    _jax.config.update("jax_compilation_cache_dir", "/tmp/jax_cache")
    _jax.config.update("jax_persistent_cache_min_compile_time_secs", 2)
    _jax.config.update("jax_persistent_cache_min_entry_size_bytes", 0)
except Exception:
    pass

import concourse.bass as bass
import concourse.bacc as bacc
import concourse.mybir as mybir
import concourse.tile as tile
from concourse.bass import ts
from concourse.bass_utils import run_bass_kernel_spmd

N_CORES = 8
B = 32768
B_LOC = B // N_CORES  # 4096
NB = 512  # batch tile (matmul moving dim)
BT = B_LOC // NB  # 8 batch tiles per core
P = 128
F_IN = 256  # 16*4*4 flattened input features
K1 = F_IN // P  # 2
H1 = 3712  # 3648 padded up to 29*128
M1 = H1 // P  # 29
H2 = 512
M2 = H2 // P  # 4
H3 = 128
NA = 4  # num actions

F32 = mybir.dt.float32
F32R = mybir.dt.float32r

KERNELS = [(1, 2), (2, 1), (1, 3), (3, 1), (1, 4), (4, 1), (2, 2)]
OFFSETS = np.cumsum([0] + [64 * (5 - kh) * (5 - kw) for kh, kw in KERNELS])

# L1 chunk plan: 29 chunks x (conv idx, layout, [2 output positions or None]).
# Chunks chosen so each chunk's input support lies in as few 128-row layout
# halves as possible (see module docstring).
_L1_PLAN = [
    (0, "A", [(0, 0), (0, 1)]), (0, "A", [(0, 2), (1, 0)]),
    (0, "A", [(1, 1), (1, 2)]), (0, "A", [(2, 0), (2, 1)]),
    (0, "A", [(2, 2), (3, 0)]), (0, "A", [(3, 1), (3, 2)]),
    (1, "B", [(0, 0), (1, 0)]), (1, "B", [(2, 0), (0, 1)]),
    (1, "B", [(1, 1), (2, 1)]), (1, "B", [(0, 2), (1, 2)]),
    (1, "B", [(2, 2), (0, 3)]), (1, "B", [(1, 3), (2, 3)]),
    (2, "A", [(0, 0), (0, 1)]), (2, "A", [(1, 0), (1, 1)]),
    (2, "A", [(2, 0), (2, 1)]), (2, "A", [(3, 0), (3, 1)]),
    (3, "B", [(0, 0), (1, 0)]), (3, "B", [(0, 1), (1, 1)]),
    (3, "B", [(0, 2), (1, 2)]), (3, "B", [(0, 3), (1, 3)]),
    (4, "A", [(0, 0), (1, 0)]), (4, "A", [(2, 0), (3, 0)]),
    (5, "B", [(0, 0), (0, 1)]), (5, "B", [(0, 2), (0, 3)]),
    (6, "A", [(0, 0), (0, 1)]), (6, "B", [(0, 2), (1, 2)]),
    (6, "B", [(1, 0), (2, 0)]), (6, "A", [(2, 1), (2, 2)]),
    (6, "A", [(1, 1), None]),
]
assert len(_L1_PLAN) == M1


def _pos(lay, h, w):
    return h * 4 + w if lay == "A" else w * 4 + h


def _l1_mm_table():
    """Per chunk: list of (layout, half, mm_index); mm_index into Wmm."""
    table = []
    mm_idx = 0
    for ci, lay, grp in _L1_PLAN:
        kh, kw = KERNELS[ci]
        halves = set()
        for o in grp:
            if o is None:
                continue
            pi, pj = o
            for dh in range(kh):
                for dw in range(kw):
                    halves.add(_pos(lay, pi + dh, pj + dw) // 8)
        entry = [(lay, tl, mm_idx + s) for s, tl in enumerate(sorted(halves))]
        mm_idx += len(entry)
        table.append(entry)
    return table, mm_idx


_L1_MM_TABLE, L1_NMM = _l1_mm_table()  # 31 matmuls


def _build_l1_weights(ws, bs):
    """Wmm [L1_NMM,128,128] (K-row, M-col), per-chunk bias [M1,128], perm."""
    Wmm = np.zeros((L1_NMM, P, P), np.float32)
    bch = np.zeros((M1, P), np.float32)
    perm = np.full(H1, -1, np.int64)
    for j, ((ci, lay, grp), entry) in enumerate(zip(_L1_PLAN, _L1_MM_TABLE)):
        kh, kw = KERNELS[ci]
        oh, ow = 5 - kh, 5 - kw
        half_to_mm = {tl: mi for (_, tl, mi) in entry}
        for sl, o in enumerate(grp):
            if o is None:
                continue
            pi, pj = o
            cols = slice(sl * 64, sl * 64 + 64)
            bch[j, cols] = bs[ci]
            perm[j * P + sl * 64 : j * P + sl * 64 + 64] = (
                OFFSETS[ci] + np.arange(64) * oh * ow + pi * ow + pj
            )
            for dh in range(kh):
                for dw in range(kw):
                    pos = _pos(lay, pi + dh, pj + dw)
                    tl = pos // 8
                    row = (pos - tl * 8) * 16
                    # [16 c, 64 oc] block
                    Wmm[half_to_mm[tl], row : row + 16, cols] = ws[ci][
                        :, :, dh, dw
                    ].T
    return Wmm, bch, perm


def _x_row_perm():
    """idxA/idxB: new row r -> original flat-x feature index."""
    idx = {}
    for lay in ("A", "B"):
        a = np.empty(F_IN, np.int64)
        for r in range(F_IN):
            pos, c = r // 16, r % 16
            if lay == "A":
                h, w = pos // 4, pos % 4
            else:
                w, h = pos // 4, pos % 4
            a[r] = c * 16 + h * 4 + w
        idx[lay] = a
    return idx["A"], idx["B"]


_IDX_A, _IDX_B = _x_row_perm()

_PROGRAM_CACHE = {}


def _build_program(repeat=1):
    nc = bacc.Bacc(None, target_bir_lowering=False)
    xa_d = nc.declare_dram_parameter("xa", [K1, P, B_LOC], F32R, isOutput=False)
    xb_d = nc.declare_dram_parameter("xb", [K1, P, B_LOC], F32R, isOutput=False)
    wcs_d = nc.declare_dram_parameter("wcs", [L1_NMM, P, P], F32R, isOutput=False)
    bc_d = nc.declare_dram_parameter("bc", [P, M1], F32, isOutput=False)
    fw0_d = nc.declare_dram_parameter("fw0t", [M1, P, H2], F32R, isOutput=False)
    fb0_d = nc.declare_dram_parameter("fb0", [P, M2], F32, isOutput=False)
    fw1_d = nc.declare_dram_parameter("fw1t", [M2, P, H3], F32R, isOutput=False)
    fb1_d = nc.declare_dram_parameter("fb1", [P, 1], F32, isOutput=False)
    fw2_d = nc.declare_dram_parameter("fw2t", [P, NA], F32R, isOutput=False)
    fb2_d = nc.declare_dram_parameter("fb2", [NA, 1], F32, isOutput=False)
    out_d = nc.declare_dram_parameter("out", [NA, B_LOC], F32, isOutput=True)

    RELU = mybir.ActivationFunctionType.Relu
    ADD = mybir.AluOpType.add
    MAX = mybir.AluOpType.max

    with tile.TileContext(nc) as tc:
        with (
            tc.tile_pool(name="wpool", bufs=1) as wpool,
            tc.tile_pool(name="xpool", bufs=2) as xpool,
            tc.tile_pool(name="a1pool", bufs=1) as a1pool,
            tc.tile_pool(name="apool", bufs=2) as apool,
            tc.tile_pool(name="opool", bufs=2) as opool,
            tc.tile_pool(name="pspool", bufs=7, space="PSUM") as pspool,
            tc.tile_pool(name="ps4pool", bufs=1, space="PSUM") as ps4pool,
        ):
            # --- load all (replicated) weights once; they stay resident.
            # One tile per chunk so compute depends on each chunk's own DMA,
            # not the whole weight stream (overlaps prologue with compute).
            bc = wpool.tile([P, M1], F32)
            nc.sync.dma_start(bc[:], bc_d[:])
            fb0 = wpool.tile([P, M2], F32)
            nc.sync.dma_start(fb0[:], fb0_d[:])
            fb1 = wpool.tile([P, 1], F32)
            nc.sync.dma_start(fb1[:], fb1_d[:])
            fb2 = wpool.tile([NA, 1], F32)
            nc.sync.dma_start(fb2[:], fb2_d[:])
            wcs = []
            for m in range(L1_NMM):
                w = wpool.tile([P, P], F32R, tag=f"wcs{m}")
                nc.sync.dma_start(w[:], wcs_d[m])
                wcs.append(w)
            fw0 = []
            for m in range(M1):
                w = wpool.tile([P, H2], F32R, tag=f"fw0_{m}")
                nc.sync.dma_start(w[:], fw0_d[m])
                fw0.append(w)
            fw1 = wpool.tile([P, M2, H3], F32R)
            for m in range(M2):
                nc.sync.dma_start(fw1[:, m, :], fw1_d[m])
            fw2 = wpool.tile([P, NA], F32R)
            nc.sync.dma_start(fw2[:], fw2_d[:])

            def tail(t, a2):
                # L3: a3 = relu(fw1 @ a2 + fb1), [128, NB]
                a3 = apool.tile([P, NB], F32R, tag="a3")
                ps = pspool.tile([P, NB], F32, tag="ps")
                for k in range(M2):
                    nc.tensor.matmul(
                        ps[:],
                        fw1[:, k, :],
                        a2[:, k, :],
                        start=(k == 0),
                        stop=(k == M2 - 1),
                    )
                nc.scalar.activation(a3[:], ps[:], RELU, bias=fb1[:, 0:1])

                # L4: out = fw2 @ a3 + fb2, [4, NB]
                ps4 = ps4pool.tile([NA, NB], F32, tag="ps4")
                nc.tensor.matmul(
                    ps4[:],
                    fw2[:],
                    a3[:],
                    start=True,
                    stop=True,
                )
                ob = opool.tile([NA, NB], F32, tag="ob")
                nc.vector.tensor_scalar_add(ob[:], ps4[:], fb2[:, 0:1])
                nc.sync.dma_start(out_d[:, ts(t, NB)], ob[:])

            def body():
              pending = None
              for t in range(BT):
                xta = xpool.tile([P, K1, NB], F32R, tag="xta")
                xtb = xpool.tile([P, K1, NB], F32R, tag="xtb")
                # x loads go on the gpsimd (SWDGE) queue so they don't sit
                # behind the weight stream on the sync queue
                for k in range(K1):
                    nc.gpsimd.dma_start(xta[:, k, :], xa_d[k, :, ts(t, NB)])
                    nc.gpsimd.dma_start(xtb[:, k, :], xb_d[k, :, ts(t, NB)])

                # L1: a1 = relu(Wc @ x + bc), feature-major [H1, NB],
                # structured conv chunks (see module docstring)
                a1 = a1pool.tile([P, M1, NB], F32R, tag="a1")
                for j in range(M1):
                    entry = _L1_MM_TABLE[j]
                    ps = pspool.tile([P, NB], F32, tag="ps")
                    for s, (lay, tl, mi) in enumerate(entry):
                        xt = xta if lay == "A" else xtb
                        nc.tensor.matmul(
                            ps[:],
                            wcs[mi][:],
                            xt[:, tl, :],
                            start=(s == 0),
                            stop=(s == len(entry) - 1),
                        )
                    # split bias+relu between DVE and ACT so neither lags PE
                    if j % 2 == 0:
                        nc.scalar.activation(
                            a1[:, j, :], ps[:], RELU, bias=bc[:, j : j + 1]
                        )
                    else:
                        nc.vector.tensor_scalar(
                            a1[:, j, :], ps[:], bc[:, j : j + 1], 0.0, ADD, MAX
                        )

                # previous tile's L3/L4 slot in here: by the time PE reaches
                # them, their inputs are long since drained -> no PE stall
                if pending is not None:
                    tail(*pending)
                    pending = None

                # L2: a2 = relu(fw0 @ a1 + fb0), [512, NB]
                a2 = apool.tile([P, M2, NB], F32R, tag="a2")
                for m in range(M2):
                    ps = pspool.tile([P, NB], F32, tag="ps")
                    for k in range(M1):
                        nc.tensor.matmul(
                            ps[:],
                            fw0[k][:, ts(m, P)],
                            a1[:, k, :],
                            start=(k == 0),
                            stop=(k == M1 - 1),
                        )
                    if m % 2 == 0:
                        nc.vector.tensor_scalar(
                            a2[:, m, :], ps[:], fb0[:, m : m + 1], 0.0, ADD, MAX
                        )
                    else:
                        nc.scalar.activation(
                            a2[:, m, :], ps[:], RELU, bias=fb0[:, m : m + 1]
                        )

                pending = (t, a2)
              tail(*pending)

            if repeat == 1:
                body()
            else:
                with tc.For_i(0, repeat, 1):
                    body()

    nc.finalize()
    return nc


def pack_inputs(x, ws, bs, fw0, fb0, fw1, fb1, fw2, fb2):
    """Pack full-problem numpy inputs into the per-core DRAM in_maps."""
    x = np.asarray(x, np.float32).reshape(B, F_IN)
    ws = [np.asarray(w, np.float32) for w in ws]
    bs = [np.asarray(b, np.float32) for b in bs]
    fw0 = np.asarray(fw0, np.float32)
    fb0 = np.asarray(fb0, np.float32)
    fw1 = np.asarray(fw1, np.float32)
    fb1 = np.asarray(fb1, np.float32)
    fw2 = np.asarray(fw2, np.float32)
    fb2 = np.asarray(fb2, np.float32)

    Wmm, bch, perm = _build_l1_weights(ws, bs)

    # fw0 with columns permuted to the structured L1 feature order
    fw0_perm = np.zeros((H2, H1), np.float32)
    valid = perm >= 0
    fw0_perm[:, valid] = fw0[:, perm[valid]]

    wcs_h = np.ascontiguousarray(Wmm)  # [NMM, 128(K), 128(M)]
    bc_h = np.ascontiguousarray(bch.T)  # [128, M1]
    fw0t_h = np.ascontiguousarray(fw0_perm.T).reshape(M1, P, H2)
    fb0_h = np.ascontiguousarray(fb0.reshape(M2, P).T)
    fw1t_h = np.ascontiguousarray(fw1.T).reshape(M2, P, H3)
    fb1_h = np.ascontiguousarray(fb1.reshape(1, P).T)
    fw2t_h = np.ascontiguousarray(fw2.T)  # [128, 4]
    fb2_h = np.ascontiguousarray(fb2.reshape(1, NA).T)  # [4, 1]

    shared = {
        "wcs": wcs_h,
        "bc": bc_h,
        "fw0t": fw0t_h,
        "fb0": fb0_h,
        "fw1t": fw1t_h,
        "fb1": fb1_h,
        "fw2t": fw2t_h,
        "fb2": fb2_h,
    }
    in_maps = []
    for i in range(N_CORES):
        shard_t = x[i * B_LOC : (i + 1) * B_LOC].T  # [256, B_LOC] view
        xa = np.ascontiguousarray(shard_t[_IDX_A]).reshape(K1, P, B_LOC)
        xb = np.ascontiguousarray(shard_t[_IDX_B]).reshape(K1, P, B_LOC)
        in_maps.append({"xa": xa, "xb": xb, **shared})
    return in_maps


def kernel(x, w0, b0, w1, b1, w2, b2, w3, b3, w4, b4, w5, b5, w6, b6,
           fw0, fb0, fw1, fb1, fw2, fb2):
    in_maps = pack_inputs(
        x, (w0, w1, w2, w3, w4, w5, w6), (b0, b1, b2, b3, b4, b5, b6),
        fw0, fb0, fw1, fb1, fw2, fb2,
    )
    if "nc" not in _PROGRAM_CACHE:
        _PROGRAM_CACHE["nc"] = _build_program()
    nc = _PROGRAM_CACHE["nc"]

    res = run_bass_kernel_spmd(nc, in_maps, list(range(N_CORES)))
    out = np.concatenate([r["out"] for r in res.results], axis=1)  # [4, B]
    return np.ascontiguousarray(out.T)


# revision 15
# speedup vs baseline: 1.3071x; 1.1620x over previous
"""Trainium2 Bass kernel for nn_CNNQNetwork (dense_cnn).

The reference network applies 7 small convs to a fixed 4x4x16 input with
VALID padding, concatenates the relu'd outputs (3648 features), then a
3-layer MLP (3648 -> 512 -> 128 -> 4).  Because the spatial input is tiny
and fixed, the conv+concat stage is one linear map of the flattened input,
so the device kernel is a 4-layer MLP:

    256 -> 3712 (relu, padded) -> 512 (relu) -> 128 (relu) -> 4

Sharding: pure data parallel over 8 NeuronCores (4096 samples each),
weights replicated.  Activations are kept feature-major on-chip
(partitions = features, free dim = batch) so every layer is a natural
lhsT.T @ rhs matmul with no on-chip transposes; the host pre-transposes x
and post-transposes the [4, B] output.  Matmuls run in float32r (fp32
operands truncated to ~fp22 at the PE) which streams at full PE rate for
moving dims >= 256, with fp32 PSUM accumulation.

First-layer structure trick: a dense 256-contraction needs 2 matmuls per
128-feature output chunk.  Instead, x is shipped in two spatial layouts
(A: row = (h*4+w)*16+c, B: row = (w*4+h)*16+c).  Output features are
permuted (absorbed into fw0's columns) so each 128-feature chunk covers 2
conv output positions whose input support fits inside ONE 128-row half of
a layout; the chunk is then a single K=128 matmul against that half with
zero-padded weight rows.  Only the 2x2 conv's middle row straddles both
halves (2 matmuls).  31 matmuls/batch-tile instead of 58.
"""

import numpy as np

try:
    # Persistent XLA executable cache: skips the multi-minute NeuronCC
    # compile on repeat runs of the identical program in this container.
    import jax as _jax

    _jax.config.update("jax_compilation_cache_dir", "/tmp/jax_cache")
    _jax.config.update("jax_persistent_cache_min_compile_time_secs", 2)
    _jax.config.update("jax_persistent_cache_min_entry_size_bytes", 0)
except Exception:
    pass

import concourse.bass as bass
import concourse.bacc as bacc
import concourse.mybir as mybir
import concourse.tile as tile
from concourse.bass import ts
from concourse.bass_utils import run_bass_kernel_spmd

N_CORES = 8
B = 32768
B_LOC = B // N_CORES  # 4096
NB = 512  # batch tile (matmul moving dim)
BT = B_LOC // NB  # 8 batch tiles per core
P = 128
F_IN = 256  # 16*4*4 flattened input features
K1 = F_IN // P  # 2
H1 = 3712  # 3648 padded up to 29*128
M1 = H1 // P  # 29
H2 = 512
M2 = H2 // P  # 4
H3 = 128
NA = 4  # num actions

F32 = mybir.dt.float32
F32R = mybir.dt.float32r

KERNELS = [(1, 2), (2, 1), (1, 3), (3, 1), (1, 4), (4, 1), (2, 2)]
OFFSETS = np.cumsum([0] + [64 * (5 - kh) * (5 - kw) for kh, kw in KERNELS])

# L1 chunk plan: 29 chunks x (conv idx, layout, [2 output positions or None]).
# Chunks chosen so each chunk's input support lies in as few 128-row layout
# halves as possible (see module docstring).
_L1_PLAN = [
    (0, "A", [(0, 0), (0, 1)]), (0, "A", [(0, 2), (1, 0)]),
    (0, "A", [(1, 1), (1, 2)]), (0, "A", [(2, 0), (2, 1)]),
    (0, "A", [(2, 2), (3, 0)]), (0, "A", [(3, 1), (3, 2)]),
    (1, "B", [(0, 0), (1, 0)]), (1, "B", [(2, 0), (0, 1)]),
    (1, "B", [(1, 1), (2, 1)]), (1, "B", [(0, 2), (1, 2)]),
    (1, "B", [(2, 2), (0, 3)]), (1, "B", [(1, 3), (2, 3)]),
    (2, "A", [(0, 0), (0, 1)]), (2, "A", [(1, 0), (1, 1)]),
    (2, "A", [(2, 0), (2, 1)]), (2, "A", [(3, 0), (3, 1)]),
    (3, "B", [(0, 0), (1, 0)]), (3, "B", [(0, 1), (1, 1)]),
    (3, "B", [(0, 2), (1, 2)]), (3, "B", [(0, 3), (1, 3)]),
    (4, "A", [(0, 0), (1, 0)]), (4, "A", [(2, 0), (3, 0)]),
    (5, "B", [(0, 0), (0, 1)]), (5, "B", [(0, 2), (0, 3)]),
    (6, "A", [(0, 0), (0, 1)]), (6, "B", [(0, 2), (1, 2)]),
    (6, "B", [(1, 0), (2, 0)]), (6, "A", [(2, 1), (2, 2)]),
    (6, "A", [(1, 1), None]),
]
assert len(_L1_PLAN) == M1


def _pos(lay, h, w):
    return h * 4 + w if lay == "A" else w * 4 + h


def _l1_mm_table():
    """Per chunk: list of (layout, half, mm_index); mm_index into Wmm."""
    table = []
    mm_idx = 0
    for ci, lay, grp in _L1_PLAN:
        kh, kw = KERNELS[ci]
        halves = set()
        for o in grp:
            if o is None:
                continue
            pi, pj = o
            for dh in range(kh):
                for dw in range(kw):
                    halves.add(_pos(lay, pi + dh, pj + dw) // 8)
        entry = [(lay, tl, mm_idx + s) for s, tl in enumerate(sorted(halves))]
        mm_idx += len(entry)
        table.append(entry)
    return table, mm_idx


_L1_MM_TABLE, L1_NMM = _l1_mm_table()  # 31 matmuls


def _build_l1_weights(ws, bs):
    """Wmm [L1_NMM,128,128] (K-row, M-col), per-chunk bias [M1,128], perm."""
    Wmm = np.zeros((L1_NMM, P, P), np.float32)
    bch = np.zeros((M1, P), np.float32)
    perm = np.full(H1, -1, np.int64)
    for j, ((ci, lay, grp), entry) in enumerate(zip(_L1_PLAN, _L1_MM_TABLE)):
        kh, kw = KERNELS[ci]
        oh, ow = 5 - kh, 5 - kw
        half_to_mm = {tl: mi for (_, tl, mi) in entry}
        for sl, o in enumerate(grp):
            if o is None:
                continue
            pi, pj = o
            cols = slice(sl * 64, sl * 64 + 64)
            bch[j, cols] = bs[ci]
            perm[j * P + sl * 64 : j * P + sl * 64 + 64] = (
                OFFSETS[ci] + np.arange(64) * oh * ow + pi * ow + pj
            )
            for dh in range(kh):
                for dw in range(kw):
                    pos = _pos(lay, pi + dh, pj + dw)
                    tl = pos // 8
                    row = (pos - tl * 8) * 16
                    # [16 c, 64 oc] block
                    Wmm[half_to_mm[tl], row : row + 16, cols] = ws[ci][
                        :, :, dh, dw
                    ].T
    return Wmm, bch, perm


def _x_row_perm():
    """idxA/idxB: new row r -> original flat-x feature index."""
    idx = {}
    for lay in ("A", "B"):
        a = np.empty(F_IN, np.int64)
        for r in range(F_IN):
            pos, c = r // 16, r % 16
            if lay == "A":
                h, w = pos // 4, pos % 4
            else:
                w, h = pos // 4, pos % 4
            a[r] = c * 16 + h * 4 + w
        idx[lay] = a
    return idx["A"], idx["B"]


_IDX_A, _IDX_B = _x_row_perm()

_PROGRAM_CACHE = {}


def _build_program(repeat=1):
    nc = bacc.Bacc(None, target_bir_lowering=False)
    xa_d = nc.declare_dram_parameter("xa", [K1, P, B_LOC], F32R, isOutput=False)
    xb_d = nc.declare_dram_parameter("xb", [K1, P, B_LOC], F32R, isOutput=False)
    wcs_d = nc.declare_dram_parameter("wcs", [L1_NMM, P, P], F32R, isOutput=False)
    bc_d = nc.declare_dram_parameter("bc", [P, M1], F32, isOutput=False)
    fw0_d = nc.declare_dram_parameter("fw0t", [M1, P, H2], F32R, isOutput=False)
    fb0_d = nc.declare_dram_parameter("fb0", [P, M2], F32, isOutput=False)
    fw1_d = nc.declare_dram_parameter("fw1t", [M2, P, H3], F32R, isOutput=False)
    fb1_d = nc.declare_dram_parameter("fb1", [P, 1], F32, isOutput=False)
    fw2_d = nc.declare_dram_parameter("fw2t", [P, NA], F32R, isOutput=False)
    fb2_d = nc.declare_dram_parameter("fb2", [NA, 1], F32, isOutput=False)
    out_d = nc.declare_dram_parameter("out", [NA, B_LOC], F32, isOutput=True)

    RELU = mybir.ActivationFunctionType.Relu
    ADD = mybir.AluOpType.add
    MAX = mybir.AluOpType.max

    with tile.TileContext(nc) as tc:
        with (
            tc.tile_pool(name="wpool", bufs=1) as wpool,
            tc.tile_pool(name="xpool", bufs=2) as xpool,
            tc.tile_pool(name="a1pool", bufs=1) as a1pool,
            tc.tile_pool(name="apool", bufs=2) as apool,
            tc.tile_pool(name="opool", bufs=2) as opool,
            tc.tile_pool(name="pspool", bufs=3, space="PSUM") as pspool,
            tc.tile_pool(name="ps1pool", bufs=1, space="PSUM") as ps1pool,
            tc.tile_pool(name="ps4pool", bufs=1, space="PSUM") as ps4pool,
        ):
            # --- load all (replicated) weights once; they stay resident.
            # One tile per chunk so compute depends on each chunk's own DMA,
            # not the whole weight stream (overlaps prologue with compute).
            bc = wpool.tile([P, M1], F32)
            nc.sync.dma_start(bc[:], bc_d[:])
            fb0 = wpool.tile([P, M2], F32)
            nc.sync.dma_start(fb0[:], fb0_d[:])
            fb1 = wpool.tile([P, 1], F32)
            nc.sync.dma_start(fb1[:], fb1_d[:])
            fb2 = wpool.tile([NA, 1], F32)
            nc.sync.dma_start(fb2[:], fb2_d[:])
            wcs = []
            for m in range(L1_NMM):
                w = wpool.tile([P, P], F32R, tag=f"wcs{m}")
                nc.sync.dma_start(w[:], wcs_d[m])
                wcs.append(w)
            fw0 = []
            for m in range(M1):
                w = wpool.tile([P, H2], F32R, tag=f"fw0_{m}")
                nc.sync.dma_start(w[:], fw0_d[m])
                fw0.append(w)
            fw1 = wpool.tile([P, M2, H3], F32R)
            for m in range(M2):
                nc.sync.dma_start(fw1[:, m, :], fw1_d[m])
            fw2 = wpool.tile([P, NA], F32R)
            nc.sync.dma_start(fw2[:], fw2_d[:])

            def tail(t, a2):
                # L3: a3 = relu(fw1 @ a2 + fb1), [128, NB]
                a3 = apool.tile([P, NB], F32R, tag="a3")
                ps = ps1pool.tile([P, NB], F32, tag="ps1")
                for k in range(M2):
                    nc.tensor.matmul(
                        ps[:],
                        fw1[:, k, :],
                        a2[:, k, :],
                        start=(k == 0),
                        stop=(k == M2 - 1),
                    )
                nc.scalar.activation(a3[:], ps[:], RELU, bias=fb1[:, 0:1])

                # L4: out = fw2 @ a3 + fb2, [4, NB]
                ps4 = ps4pool.tile([NA, NB], F32, tag="ps4")
                nc.tensor.matmul(
                    ps4[:],
                    fw2[:],
                    a3[:],
                    start=True,
                    stop=True,
                )
                ob = opool.tile([NA, NB], F32, tag="ob")
                nc.vector.tensor_scalar_add(ob[:], ps4[:], fb2[:, 0:1])
                nc.sync.dma_start(out_d[:, ts(t, NB)], ob[:])

            def body():
              pending = None
              for t in range(BT):
                xta = xpool.tile([P, K1, NB], F32R, tag="xta")
                xtb = xpool.tile([P, K1, NB], F32R, tag="xtb")
                # x loads go on the gpsimd (SWDGE) queue so they don't sit
                # behind the weight stream on the sync queue
                for k in range(K1):
                    nc.gpsimd.dma_start(xta[:, k, :], xa_d[k, :, ts(t, NB)])
                    nc.gpsimd.dma_start(xtb[:, k, :], xb_d[k, :, ts(t, NB)])

                # L1: a1 = relu(Wc @ x + bc), feature-major [H1, NB],
                # structured conv chunks (see module docstring).  Adjacent
                # chunk pairs come from the same conv, so they share one
                # per-partition bias vector and can drain (bias+relu) in a
                # single op on a 2-bank PSUM tile — 15 drains instead of 29.
                a1 = a1pool.tile([P, M1, NB], F32R, tag="a1")
                for jp in range(M1 // 2):
                    jA = 2 * jp
                    ps = pspool.tile([P, 2, NB], F32, tag="ps2")
                    for idx in range(2):
                        entry = _L1_MM_TABLE[jA + idx]
                        for s, (lay, tl, mi) in enumerate(entry):
                            xt = xta if lay == "A" else xtb
                            nc.tensor.matmul(
                                ps[:, idx, :],
                                wcs[mi][:],
                                xt[:, tl, :],
                                start=(s == 0),
                                stop=(s == len(entry) - 1),
                            )
                    # split bias+relu between DVE and ACT so neither lags PE
                    if jp % 2 == 0:
                        nc.scalar.activation(
                            a1[:, jA : jA + 2, :], ps[:], RELU,
                            bias=bc[:, jA : jA + 1],
                        )
                    else:
                        nc.vector.tensor_scalar(
                            a1[:, jA : jA + 2, :], ps[:], bc[:, jA : jA + 1],
                            0.0, ADD, MAX,
                        )
                # last (odd) chunk drains alone
                j = M1 - 1
                entry = _L1_MM_TABLE[j]
                ps = ps1pool.tile([P, NB], F32, tag="ps1")
                for s, (lay, tl, mi) in enumerate(entry):
                    xt = xta if lay == "A" else xtb
                    nc.tensor.matmul(
                        ps[:],
                        wcs[mi][:],
                        xt[:, tl, :],
                        start=(s == 0),
                        stop=(s == len(entry) - 1),
                    )
                nc.scalar.activation(
                    a1[:, j, :], ps[:], RELU, bias=bc[:, j : j + 1]
                )

                # previous tile's L3/L4 slot in here: by the time PE reaches
                # them, their inputs are long since drained -> no PE stall
                if pending is not None:
                    tail(*pending)
                    pending = None

                # L2: a2 = relu(fw0 @ a1 + fb0), [512, NB]
                a2 = apool.tile([P, M2, NB], F32R, tag="a2")
                for m in range(M2):
                    ps2 = pspool.tile([P, 2, NB], F32, tag="ps2")
                    ps = ps2[:, 0, :]
                    for k in range(M1):
                        nc.tensor.matmul(
                            ps[:],
                            fw0[k][:, ts(m, P)],
                            a1[:, k, :],
                            start=(k == 0),
                            stop=(k == M1 - 1),
                        )
                    if m % 2 == 0:
                        nc.vector.tensor_scalar(
                            a2[:, m, :], ps[:], fb0[:, m : m + 1], 0.0, ADD, MAX
                        )
                    else:
                        nc.scalar.activation(
                            a2[:, m, :], ps[:], RELU, bias=fb0[:, m : m + 1]
                        )

                pending = (t, a2)
              tail(*pending)

            if repeat == 1:
                body()
            else:
                with tc.For_i(0, repeat, 1):
                    body()

    nc.finalize()
    return nc


def pack_inputs(x, ws, bs, fw0, fb0, fw1, fb1, fw2, fb2):
    """Pack full-problem numpy inputs into the per-core DRAM in_maps."""
    x = np.asarray(x, np.float32).reshape(B, F_IN)
    ws = [np.asarray(w, np.float32) for w in ws]
    bs = [np.asarray(b, np.float32) for b in bs]
    fw0 = np.asarray(fw0, np.float32)
    fb0 = np.asarray(fb0, np.float32)
    fw1 = np.asarray(fw1, np.float32)
    fb1 = np.asarray(fb1, np.float32)
    fw2 = np.asarray(fw2, np.float32)
    fb2 = np.asarray(fb2, np.float32)

    Wmm, bch, perm = _build_l1_weights(ws, bs)

    # fw0 with columns permuted to the structured L1 feature order
    fw0_perm = np.zeros((H2, H1), np.float32)
    valid = perm >= 0
    fw0_perm[:, valid] = fw0[:, perm[valid]]

    wcs_h = np.ascontiguousarray(Wmm)  # [NMM, 128(K), 128(M)]
    bc_h = np.ascontiguousarray(bch.T)  # [128, M1]
    fw0t_h = np.ascontiguousarray(fw0_perm.T).reshape(M1, P, H2)
    fb0_h = np.ascontiguousarray(fb0.reshape(M2, P).T)
    fw1t_h = np.ascontiguousarray(fw1.T).reshape(M2, P, H3)
    fb1_h = np.ascontiguousarray(fb1.reshape(1, P).T)
    fw2t_h = np.ascontiguousarray(fw2.T)  # [128, 4]
    fb2_h = np.ascontiguousarray(fb2.reshape(1, NA).T)  # [4, 1]

    shared = {
        "wcs": wcs_h,
        "bc": bc_h,
        "fw0t": fw0t_h,
        "fb0": fb0_h,
        "fw1t": fw1t_h,
        "fb1": fb1_h,
        "fw2t": fw2t_h,
        "fb2": fb2_h,
    }
    in_maps = []
    for i in range(N_CORES):
        shard_t = x[i * B_LOC : (i + 1) * B_LOC].T  # [256, B_LOC] view
        xa = np.ascontiguousarray(shard_t[_IDX_A]).reshape(K1, P, B_LOC)
        xb = np.ascontiguousarray(shard_t[_IDX_B]).reshape(K1, P, B_LOC)
        in_maps.append({"xa": xa, "xb": xb, **shared})
    return in_maps


def kernel(x, w0, b0, w1, b1, w2, b2, w3, b3, w4, b4, w5, b5, w6, b6,
           fw0, fb0, fw1, fb1, fw2, fb2):
    in_maps = pack_inputs(
        x, (w0, w1, w2, w3, w4, w5, w6), (b0, b1, b2, b3, b4, b5, b6),
        fw0, fb0, fw1, fb1, fw2, fb2,
    )
    if "nc" not in _PROGRAM_CACHE:
        _PROGRAM_CACHE["nc"] = _build_program()
    nc = _PROGRAM_CACHE["nc"]

    res = run_bass_kernel_spmd(nc, in_maps, list(range(N_CORES)))
    out = np.concatenate([r["out"] for r in res.results], axis=1)  # [4, B]
    return np.ascontiguousarray(out.T)
